# revision 14
# baseline (speedup 1.0000x reference)
"""InteractionMapInit Trainium2 kernel.

out[i, j, :] = tanh( (X@Wt + bt)[i] - (Dft@Wd + bd)[j] + dnorm[i, j] )  if seg_res[i] == seg_atom[j]
             = 0                                                        otherwise

One block (DT pair) per core, SPMD over 8 cores; the host scatters the 8
dense blocks into the zeros [NR, NA, H] output (the mask is block-diagonal,
so everything off-block is zero and never touches the device).

The O(NR*TD*H) target linear and the O(NR*NA*H) interaction-map
materialization run on the device; the O(NR+NA)-sized side quantities
(drug linear df, per-block distance normalization dnorm -- ~1% of the
FLOPs) are host prep, shipped as small fp16 constants:

  lhsT2 [128, Rp] = [dnorm^T (Ap rows); ones; zeros]         (fp16)
  R2D   [128, AH] = [kron(I_Ap, 1_H); bt - df flat; zeros]   (fp16)

Device program per core (Rp padded rows, Ap padded atoms):
  - tfT [H, Rp] = Wt.T @ X.T   f32r matmuls (N>=256), cast to fp16
  - per 128-row tile, 3-bank PSUM groups (1536 cols), fp16 matmuls:
      mm1 (lhsT = tfT slice [128,128], rhs = I4 tiled identity) -> + tf[i,h]
      mm2 (lhsT = lhsT2 slice [128,128], rhs = R2D slice)       -> + dnorm[i,j]
                                                                   + bt[h] - df[j,h]
      ACT tanh PSUM -> fp16 SBUF, DMA out (rows contiguous in DRAM)
  - fp16 operand/output precision costs ~2.5e-3 abs err (gate is 2e-2);
    fp16 matmuls stream 1 col/cycle vs ~3 for f32r and enable FWL
  - only the valid Rl rows of the last row tile are stored

Padding: features zero-padded; dnorm rows/cols beyond the block replicate
edge values (harmless, discarded on the host).
"""

import numpy as np

NR, NA, TD, DD, H, B = 3200, 320, 512, 128, 128, 8
NCORES = 8
P = 128

_last_results = None


def _host_prep(target_feature, drug_feature, target_pos, drug_pos,
               Wt, bt, Wd, bd, seg_res, seg_atom):
    f32 = np.float32
    X = np.ascontiguousarray(np.asarray(target_feature, f32))
    Dft = np.ascontiguousarray(np.asarray(drug_feature, f32))
    tp = np.asarray(target_pos, f32)
    dp = np.asarray(drug_pos, f32)
    Wt = np.ascontiguousarray(np.asarray(Wt, f32))
    Wd = np.ascontiguousarray(np.asarray(Wd, f32))
    bt = np.asarray(bt, f32).reshape(1, H)
    bd = np.asarray(bd, f32).reshape(1, H)
    seg_res = np.asarray(seg_res)
    seg_atom = np.asarray(seg_atom)

    r0 = np.searchsorted(seg_res, np.arange(B), side="left")
    r1 = np.searchsorted(seg_res, np.arange(B), side="right")
    a0 = np.searchsorted(seg_atom, np.arange(B), side="left")
    a1 = np.searchsorted(seg_atom, np.arange(B), side="right")
    r_cnt = (r1 - r0).astype(int)
    a_cnt = (a1 - a0).astype(int)

    Rp = max(P, int(-(-max(r_cnt) // P)) * P)
    Ap = max(4, int(-(-max(a_cnt) // 4)) * 4)
    assert Ap + 1 <= P, f"block atom count too large: {max(a_cnt)}"

    AH = Ap * H
    KRON = np.kron(np.eye(Ap, dtype=f32), np.ones((1, H), f32))
    I4 = np.ascontiguousarray(
        np.tile(np.eye(P, dtype=np.float16), (1, 4)))

    in_maps = []
    for c in range(B):
        rc, ac = r_cnt[c], a_cnt[c]
        XT = np.zeros((TD, Rp), f32)
        DFT = np.zeros((DD, Ap), f32)
        tpp = np.zeros((Rp, 3), f32)
        dpp = np.zeros((Ap, 3), f32)
        if rc > 0:
            XT[:, :rc] = X[r0[c]:r1[c]].T
            tpp[:rc] = tp[r0[c]:r1[c]]
            tpp[rc:] = tp[r1[c] - 1]
        if ac > 0:
            DFT[:, :ac] = Dft[a0[c]:a1[c]].T
            dpp[:ac] = dp[a0[c]:a1[c]]
            dpp[ac:] = dp[a1[c] - 1]

        # per-block distance normalization (O(Rp*Ap) -- host prep).
        # positions are edge-replicated so padded entries replicate real
        # distances and leave min/max unchanged.
        D = np.linalg.norm(tpp[:, None, :] - dpp[None, :, :], axis=-1)
        if rc > 0 and ac > 0:
            dmin, dmax = float(D.min()), float(D.max())
        else:
            dmin, dmax = 0.0, 1.0
        denom = (dmax - dmin) if dmax > dmin else 1.0
        LH2 = np.zeros((P, Rp), f32)
        LH2[:Ap, :] = ((D - dmin) / denom).T
        LH2[Ap, :] = 1.0

        # drug-side linear is tiny -> host, baked into R2D's ones-row
        R2D = np.zeros((P, AH), f32)
        R2D[:Ap, :] = KRON
        R2D[Ap, :] = (np.tile(bt, (Ap, 1)) - (DFT.T @ Wd + bd)).reshape(-1)

        in_maps.append({
            "xt": np.ascontiguousarray(XT.astype(np.float16)),
            "wt": np.ascontiguousarray(Wt.astype(np.float16)),
            "i4": I4,
            "lh2": np.ascontiguousarray(LH2.astype(np.float16)),
            "r2d": np.ascontiguousarray(R2D.astype(np.float16)),
        })

    Rl = int(max(r_cnt)) - (Rp // P - 1) * P   # valid rows in the last tile
    meta = dict(r0=r0, a0=a0, r_cnt=r_cnt, a_cnt=a_cnt, Rp=Rp, Ap=Ap, Rl=Rl)
    return in_maps, meta


def build_bass(Rp, Ap, Rl=None):
    if Rl is None:
        Rl = P
    from contextlib import ExitStack

    import concourse.bacc as bacc
    import concourse.mybir as mybir
    import concourse.tile as tile

    F32 = mybir.dt.float32
    F32R = mybir.dt.float32r
    F16 = mybir.dt.float16
    AF = mybir.ActivationFunctionType

    K_TD = TD // P        # 4 contraction chunks for the target linear
    RT = Rp // P          # 128-row tiles
    NCH = Ap // 4         # 512-wide psum chunks (4 atoms x H)
    AH = Ap * H
    # 512-col chunks per psum group: 3 banks x 2 bufs + 2 tf banks = 8
    GRP = next(g for g in (3, 2, 1) if NCH % g == 0)
    NG = NCH // GRP       # psum groups per row tile

    nc = bacc.Bacc("TRN2", target_bir_lowering=False, debug=False,
                   num_devices=NCORES)

    xt_d = nc.dram_tensor("xt", [TD, Rp], F16, kind="ExternalInput").ap()
    wt_d = nc.dram_tensor("wt", [TD, H], F16, kind="ExternalInput").ap()
    i4_d = nc.dram_tensor("i4", [P, 512], F16, kind="ExternalInput").ap()
    lh2_d = nc.dram_tensor("lh2", [P, Rp], F16, kind="ExternalInput").ap()
    r2d_d = nc.dram_tensor("r2d", [P, AH], F16, kind="ExternalInput").ap()
    out_d = nc.dram_tensor("out", [Rp, AH], F16, kind="ExternalOutput").ap()

    with tile.TileContext(nc) as tc, ExitStack() as ctx:
        singles = ctx.enter_context(tc.tile_pool(name="singles", bufs=1))
        psum = ctx.enter_context(tc.tile_pool(name="psum", bufs=2, space="PSUM"))
        pspro = ctx.enter_context(tc.tile_pool(name="pspro", bufs=2, space="PSUM"))
        outs = ctx.enter_context(tc.tile_pool(name="outs", bufs=4))

        # -------- PE warm-up: the HAM clock gate needs ~3.4us of sustained
        # activity to unthrottle 1.2 -> 2.4 GHz. Run dummy matmuls on a
        # memset tile while the input DMAs stream, so the first real
        # matmuls (and the pipeline ramp) execute at full clock.
        wsrc = singles.tile([P, 512], F16, name="wsrc")
        nc.vector.memset(wsrc, 0.25)
        for _w in range(8):
            ps_w = pspro.tile([P, 512], F32, tag="pro", name="ps_w")
            nc.tensor.matmul(ps_w, lhsT=wsrc[:, :P], rhs=wsrc,
                             start=True, stop=True)

        # ---------------- inputs to SBUF ----------------
        # SP-ring FIFO order: wt + xt tile-0 (tfT pipeline start), the small
        # fp16 constants, r2d slice 0, xt rest, r2d rest
        wt_sb = singles.tile([P, K_TD, H], F16, name="wt_sb")
        nc.sync.dma_start(out=wt_sb, in_=wt_d.rearrange("(k p) h -> p k h", p=P))
        xt_sb = singles.tile([P, K_TD, Rp], F16, name="xt_sb")
        xt_r = xt_d.rearrange("(k p) i -> p k i", p=P)
        nc.sync.dma_start(out=xt_sb[:, :, :P], in_=xt_r[:, :, :P])
        r2d_sb = singles.tile([P, AH], F16, name="r2d_sb")
        nc.sync.dma_start(out=r2d_sb[:, :512 * GRP], in_=r2d_d[:, :512 * GRP])
        nc.sync.dma_start(out=xt_sb[:, :, P:], in_=xt_r[:, :, P:])
        # second HWDGE ring (ACT-issued): constants + remaining r2d slices
        # stream in parallel with the SP ring
        i4_sb = singles.tile([P, 512], F16, name="i4_sb")
        nc.scalar.dma_start(out=i4_sb, in_=i4_d)
        lhsT2 = singles.tile([P, Rp], F16, name="lhsT2")
        nc.scalar.dma_start(out=lhsT2, in_=lh2_d)
        for j in range(GRP, NCH, GRP):
            nc.scalar.dma_start(out=r2d_sb[:, 512 * j:512 * (j + GRP)],
                                in_=r2d_d[:, 512 * j:512 * (j + GRP)])

        tfT = singles.tile([P, Rp], F16, name="tfT")

        # ---------------- main: psum = tf - df + dnorm ; tanh ; store --------
        # tfT = Wt.T @ X.T in two f32r batches (tile 0 first for an early
        # pipeline start; N=384 batch at full f32r rate), cast to fp16
        def tf_batch(b):
            csl = slice(P, Rp) if b else slice(0, P)
            w = csl.stop - csl.start
            if w <= 0:
                return
            ps_tf = pspro.tile([P, 512], F32, tag="pro", name="ps_tf")
            for k in range(K_TD):
                nc.tensor.matmul(ps_tf[:, :w], lhsT=wt_sb[:, k, :],
                                 rhs=xt_sb[:, k, csl],
                                 start=(k == 0), stop=(k == K_TD - 1))
            nc.vector.tensor_copy(out=tfT[:, csl], in_=ps_tf[:, :w])

        tf_batch(0)
        for rt in range(RT):
            rsl = slice(P * rt, P * (rt + 1))
            ob = outs.tile([P, AH], F16, name="ob")
            for g in range(NG):
                pso = psum.tile([P, GRP * 512], F32, tag="ps", name="pso")
                # batch by stationary operand: all tf-broadcast mms first
                for c in range(GRP):
                    nc.tensor.matmul(pso[:, 512 * c:512 * (c + 1)],
                                     lhsT=tfT[:, rsl], rhs=i4_sb,
                                     start=True, stop=False)
                for c in range(GRP):
                    ch = g * GRP + c
                    nc.tensor.matmul(pso[:, 512 * c:512 * (c + 1)],
                                     lhsT=lhsT2[:, rsl],
                                     rhs=r2d_sb[:, 512 * ch:512 * (ch + 1)],
                                     start=False, stop=True)
                nc.scalar.activation(out=ob[:, 512 * GRP * g:512 * GRP * (g + 1)],
                                     in_=pso, func=AF.Tanh)
                if rt == 0 and g == 0:
                    tf_batch(1)
                if rt == 0 or rt == RT - 1:
                    # first tile: start the out stream ASAP; last tile:
                    # shorten the tail (only Rl valid rows stored)
                    gsl = slice(512 * GRP * g, 512 * GRP * (g + 1))
                    if rt == RT - 1:
                        lsl = slice(P * rt, P * rt + Rl)
                        nc.sync.dma_start(out=out_d[lsl, gsl],
                                          in_=ob[:Rl, gsl])
                    else:
                        nc.sync.dma_start(out=out_d[rsl, gsl], in_=ob[:, gsl])
            if 0 < rt < RT - 1:
                nc.sync.dma_start(out=out_d[rsl, :], in_=ob)

    nc.compile()
    return nc


_last_nc = None
_last_in_maps = None


def kernel(**inputs) -> np.ndarray:
    global _last_results, _last_nc, _last_in_maps
    in_maps, meta = _host_prep(**inputs)
    Rp, Ap = meta["Rp"], meta["Ap"]

    nc = build_bass(Rp, Ap, meta["Rl"])
    _last_nc, _last_in_maps = nc, in_maps

    from concourse.bass_utils import run_bass_kernel_spmd
    res = run_bass_kernel_spmd(nc, in_maps, core_ids=list(range(NCORES)))
    _last_results = res

    out = np.zeros((NR, NA, H), np.float32)
    for c in range(B):
        rc, ac = int(meta["r_cnt"][c]), int(meta["a_cnt"][c])
        if rc == 0 or ac == 0:
            continue
        blk = np.asarray(res.results[c]["out"], np.float32).reshape(Rp, Ap, H)
        r0, a0 = int(meta["r0"][c]), int(meta["a0"][c])
        out[r0:r0 + rc, a0:a0 + ac, :] = blk[:rc, :ac, :]
    return out
